# revision 1
# baseline (speedup 1.0000x reference)
"""Trainium2 Bass kernel for nn_CrossAttentionFusion (B=4, S=2048, D=512, H=8).

Sharding: 8 cores = 4 batches x 2 head-groups (4 heads each). Each core
receives its batch's value/key/query [2048, 512] plus its head-group's
weight rows W*[256, 512] / biases [256], and produces out [256] (the 4
heads' head_logits). Host folds the 1/sqrt(64) softmax scale into Wq/bq
and multiplies the gathered output by 8 to compensate.

Per-core math, per head h (d = head dim 64, i/j = sequence):
  qT/kT/vT [64, 2048] = W_h @ inputT (+bias)        (T layouts throughout)
  sk[i,j] = sum_d qT[d,i] kT[d,j]   (already scaled) ; sv likewise with vT
  P~ = exp(s), Z[i] = sum_j P~[i,j]  (ACT fused accumulate, no max-sub:
       scores are ~N(0,1) so exp is safely bounded)
  GT[d,j] = sum_i (q_nat[i,d]/Zk[i]) P~k[i,j] + (q_nat[i,d]/Zv[i]) P~v[i,j]
  out[d]  = sum_j (kT+vT)[d,j] * GT[d,j]            (one tensor_tensor_reduce)

This formulation needs no transpose of any S x S matrix and reduces the
final (attended*q).sum over the sequence axis for free via the GT matmul.
"""

import os
import sys

import numpy as np

if "/opt/trn_rl_repo" not in sys.path and os.path.isdir("/opt/trn_rl_repo"):
    sys.path.insert(0, "/opt/trn_rl_repo")

from contextlib import ExitStack

import concourse.bass as bass
import concourse.mybir as mybir
import concourse.tile as tile
from concourse import bacc
from concourse.masks import make_identity

B, S, D, H, HD = 4, 2048, 512, 8, 64
DG = 256  # local output dims per core (4 heads x 64)
f32 = mybir.dt.float32
f32r = mybir.dt.float32r
bf16 = mybir.dt.bfloat16
FT = mybir.ActivationFunctionType
ALU = mybir.AluOpType
AXL = mybir.AxisListType


def _r(ap):
    # fp32r view: full-rate PE matmul (fp22 internally) for 4-byte operands
    return ap.bitcast(f32r)


def build_program(phase=4, reps=1, timing=False):
    # phase: 1=projections only, 2=+scores/exp, 3=+GT matmuls, 4=full
    # reps: emit the whole computation N times (timing: marginal cost/rep)
    # timing: shrink ExternalInputs (broadcast-read) so the per-call axon
    #   transfer cost collapses; kernel work is unchanged, outputs garbage
    nc = bacc.Bacc("TRN2", target_bir_lowering=False)

    ishape = [128, D] if timing else [S, D]
    wshape = [128, D] if timing else [DG, D]
    val = nc.dram_tensor("value", ishape, f32, kind="ExternalInput")
    key = nc.dram_tensor("key", ishape, f32, kind="ExternalInput")
    qry = nc.dram_tensor("query", ishape, f32, kind="ExternalInput")
    Wv = nc.dram_tensor("Wv", wshape, f32, kind="ExternalInput")
    bv = nc.dram_tensor("bv", [DG], f32, kind="ExternalInput")
    Wk = nc.dram_tensor("Wk", wshape, f32, kind="ExternalInput")
    bk = nc.dram_tensor("bk", [DG], f32, kind="ExternalInput")
    Wq = nc.dram_tensor("Wq", wshape, f32, kind="ExternalInput")
    bq = nc.dram_tensor("bq", [DG], f32, kind="ExternalInput")
    out = nc.dram_tensor("out", [DG], f32, kind="ExternalOutput")

    with tile.TileContext(nc) as tc, ExitStack() as ctx:
        const = ctx.enter_context(tc.tile_pool(name="const", bufs=1))
        ident = const.tile([128, 128], f32)
        make_identity(nc, ident)

        for rep in range(reps):
          if rep > 0:
              tc.strict_bb_all_engine_barrier()
          with ExitStack() as rctx:
            # Persistent per-pair tensors. Pair p holds local heads (2p, 2p+1)
            # stacked on partitions: partitions [0,64) = head 2p, [64,128) = 2p+1.
            qkv = rctx.enter_context(tc.tile_pool(name=f"qkv_{rep}", bufs=1))
            qT2 = [qkv.tile([128, S], f32r, name=f"qT2_{p}_{rep}") for p in (0, 1)]
            kT2 = [qkv.tile([128, S], f32r, name=f"kT2_{p}_{rep}") for p in (0, 1)]
            vT2 = [qkv.tile([128, S], f32r, name=f"vT2_{p}_{rep}") for p in (0, 1)]
            fus = [qkv.tile([128, S], f32, name=f"fus_{p}_{rep}") for p in (0, 1)]
            qn = [qkv.tile([128, 16, 128], bf16, name=f"qn_{p}_{rep}") for p in (0, 1)]
            outsb = qkv.tile([128, 2], f32, name=f"outsb_{rep}")

            # --- weights: load [256, 512] and PE-transpose to wT [c, d] chunks ---
            wT = {}
            bias = {}
            with (
                tc.tile_pool(name=f"wnat_{rep}", bufs=2) as wnatp,
                tc.tile_pool(name=f"wps_{rep}", bufs=2, space="PSUM") as wps,
            ):
                for nm, wdram, bdram in (("v", Wv, bv), ("k", Wk, bk), ("q", Wq, bq)):
                    bt = qkv.tile([128, 2], f32, name=f"b{nm}_{rep}")
                    nc.sync.dma_start(bt, bdram[:].rearrange("(t p) -> p t", p=128))
                    bias[nm] = bt
                    wn = wnatp.tile([128, 2, D], f32, tag="wn")
                    if timing:
                        wsrc = wdram[:, :].unsqueeze(1).broadcast_to([128, 2, D])
                    else:
                        wsrc = wdram[:, :].rearrange("(t p) c -> p t c", p=128)
                    nc.sync.dma_start(wn, wsrc)
                    wt = qkv.tile([128, 4, DG], f32r, name=f"wT{nm}_{rep}")
                    wT[nm] = wt
                    for cc in range(4):
                        pt = wps.tile([128, 2, 128], f32, tag="wp")
                        for dblk in range(2):
                            nc.tensor.transpose(
                                pt[:, dblk], wn[:, dblk, cc * 128 : (cc + 1) * 128], ident
                            )
                        nc.scalar.copy(
                            wt[:, cc, :], pt.rearrange("p a b -> p (a b)")
                        )

            # --- per input tensor: load, transpose to inputT, project both pairs ---
            specs = [("v", val, vT2), ("k", key, kT2), ("q", qry, qT2)]
            for nm, dram, dstT2 in specs:
                with (
                    tc.tile_pool(name=f"inT_{nm}_{rep}", bufs=1) as tp,
                    tc.tile_pool(name=f"nat_{nm}_{rep}", bufs=3) as natp,
                    tc.tile_pool(name=f"ps_{nm}_{rep}", bufs=4, space="PSUM") as tps,
                ):
                    inT = [tp.tile([128, S], f32r, name=f"inT{nm}{cc}_{rep}") for cc in range(4)]
                    if timing:
                        drv = (
                            dram[:, :].unsqueeze(0).unsqueeze(2).broadcast_to([4, 128, 4, D])
                        )
                    else:
                        drv = dram[:, :].rearrange("(g n p) c -> g p n c", g=4, n=4, p=128)
                    for ng in range(4):
                        natg = natp.tile([128, 4, D], f32, tag="nat")
                        nc.sync.dma_start(natg, drv[ng])
                        for cc in range(4):
                            pt = tps.tile([128, 4, 128], f32, tag="tp")
                            for nn in range(4):
                                nc.tensor.transpose(
                                    pt[:, nn], natg[:, nn, cc * 128 : (cc + 1) * 128], ident
                                )
                            nc.scalar.copy(
                                inT[cc][:, ng * 512 : (ng + 1) * 512],
                                pt.rearrange("p a b -> p (a b)"),
                            )
                    for p in (0, 1):
                        for jb in range(4):
                            ps = tps.tile([128, 512], f32, tag="pj")
                            for cc in range(4):
                                nc.tensor.matmul(
                                    ps,
                                    wT[nm][:, cc, p * 128 : (p + 1) * 128],
                                    inT[cc][:, jb * 512 : (jb + 1) * 512],
                                    start=(cc == 0),
                                    stop=(cc == 3),
                                )
                            nc.vector.tensor_scalar_add(
                                dstT2[p][:, jb * 512 : (jb + 1) * 512],
                                ps,
                                bias[nm][:, p : p + 1],
                            )
                    if nm == "q":
                        # natural-layout q (bf16) for the 1/Z row scaling; one
                        # [128,128] transpose per ic yields both heads' columns
                        for p in (0, 1):
                            for icg in range(4):
                                pt = tps.tile([128, 4, 128], f32, tag="tp")
                                for k4 in range(4):
                                    ic = icg * 4 + k4
                                    nc.tensor.transpose(
                                        pt[:, k4],
                                        qT2[p][:, ic * 128 : (ic + 1) * 128].bitcast(f32),
                                        ident,
                                    )
                                nc.vector.tensor_copy(
                                    qn[p][:, icg * 4 : (icg + 1) * 4, :],
                                    pt.rearrange("p a b -> p a b"),
                                )

            for p in (0, 1):
                nc.vector.tensor_add(fus[p], kT2[p].bitcast(f32), vT2[p].bitcast(f32))

            # --- attention, one head-pair at a time ---
            for p in (0, 1):
                with (
                    tc.tile_pool(name=f"aps{p}_{rep}", bufs=1, space="PSUM") as aps,
                    tc.tile_pool(name=f"pp{p}_{rep}", bufs=4) as ppool,
                    tc.tile_pool(name=f"sm{p}_{rep}", bufs=2) as smp,
                ):
                    gt2 = aps.tile([128, S], f32, name=f"gt{p}_{rep}")
                    sc = [aps.tile([128, 1024], f32, name=f"sc{p}{h}_{rep}") for h in (0, 1)]
                    wsd = {}
                    for ic in range(16):
                        if phase < 2:
                            continue
                        for m, src in ((0, kT2[p]), (1, vT2[p])):
                            pts = [
                                ppool.tile([128, S], bf16, tag=f"pt{m}{h}", name=f"pt{m}{h}_{ic}_{rep}")
                                for h in (0, 1)
                            ]
                            zp = [
                                smp.tile([128, 2], f32, tag=f"zp{m}{h}", name=f"zp{m}{h}_{ic}_{rep}")
                                for h in (0, 1)
                            ]
                            for jh in (0, 1):
                                for jq in (0, 1):
                                    j0 = (jh * 2 + jq) * 512
                                    # h0/h1 adjacent: disjoint PE row groups run
                                    # concurrently (row packing, K=64 each)
                                    for h in (0, 1):
                                        nc.tensor.matmul(
                                            sc[h][:, jq * 512 : (jq + 1) * 512],
                                            qT2[p][
                                                64 * h : 64 * (h + 1),
                                                ic * 128 : (ic + 1) * 128,
                                            ],
                                            src[64 * h : 64 * (h + 1), j0 : j0 + 512],
                                            start=True,
                                            stop=True,
                                        )
                                for h in (0, 1):
                                    nc.scalar.activation(
                                        pts[h][:, jh * 1024 : (jh + 1) * 1024],
                                        sc[h],
                                        FT.Exp,
                                        accum_out=zp[h][:, jh : jh + 1],
                                    )
                            # per-map normalizer + GT: map k's matmuls overlap map v's exps
                            for h in (0, 1):
                                zs = smp.tile([128, 1], f32, tag=f"zs{m}{h}", name=f"zs{m}{h}_{ic}_{rep}")
                                nc.vector.tensor_add(zs, zp[h][:, 0:1], zp[h][:, 1:2])
                                rs = smp.tile([128, 1], f32, tag=f"rs{m}{h}", name=f"rs{m}{h}_{ic}_{rep}")
                                nc.vector.reciprocal(rs, zs)
                                wsd[(m, h)] = smp.tile(
                                    [128, HD], bf16, tag=f"w{m}{h}", name=f"w{m}{h}_{ic}_{rep}"
                                )
                                nc.vector.tensor_scalar_mul(
                                    wsd[(m, h)],
                                    qn[p][:, ic, 64 * h : 64 * (h + 1)],
                                    rs[:, 0:1],
                                )
                            if phase < 3:
                                continue
                            for jq in range(4):
                                # h0/h1 adjacent: disjoint PE col groups (M=64)
                                for h in (0, 1):
                                    nc.tensor.matmul(
                                        gt2[64 * h : 64 * (h + 1), jq * 512 : (jq + 1) * 512],
                                        wsd[(m, h)],
                                        pts[h][:, jq * 512 : (jq + 1) * 512],
                                        start=(ic == 0 and m == 0),
                                        stop=(ic == 15 and m == 1),
                                        skip_group_check=True,
                                    )
                    scr = smp.tile([128, S], f32, tag="scr")
                    nc.vector.tensor_mul(
                        scr, gt2 if phase >= 3 else fus[p], fus[p]
                    )
                    nc.vector.tensor_reduce(
                        outsb[:, p : p + 1], scr, axis=AXL.X, op=ALU.add
                    )
                nc.sync.dma_start(
                    out[:].rearrange("(t q) -> t q", t=2)[p].unsqueeze(1),
                    outsb[:, p : p + 1],
                )

    nc.compile()
    return nc


_CACHE = {}


def _program(phase=4, reps=1, timing=False):
    key = (phase, reps, timing)
    if key not in _CACHE:
        _CACHE[key] = build_program(phase=phase, reps=reps, timing=timing)
    return _CACHE[key]


def make_in_maps(inputs):
    v = np.ascontiguousarray(np.asarray(inputs["value"], dtype=np.float32))
    k = np.ascontiguousarray(np.asarray(inputs["key"], dtype=np.float32))
    q = np.ascontiguousarray(np.asarray(inputs["query"], dtype=np.float32))
    Wv = np.asarray(inputs["Wv"], dtype=np.float32)
    Wk = np.asarray(inputs["Wk"], dtype=np.float32)
    Wq = np.asarray(inputs["Wq"], dtype=np.float32)
    bv = np.asarray(inputs["bv"], dtype=np.float32)
    bk = np.asarray(inputs["bk"], dtype=np.float32)
    bq = np.asarray(inputs["bq"], dtype=np.float32)
    in_maps = []
    for c in range(8):
        b, g = divmod(c, 2)
        sl = slice(g * DG, (g + 1) * DG)
        in_maps.append(
            {
                "value": np.ascontiguousarray(v[b]),
                "key": np.ascontiguousarray(k[b]),
                "query": np.ascontiguousarray(q[b]),
                "Wv": np.ascontiguousarray(Wv[sl]),
                "bv": np.ascontiguousarray(bv[sl]),
                "Wk": np.ascontiguousarray(Wk[sl]),
                "bk": np.ascontiguousarray(bk[sl]),
                # softmax 1/sqrt(HD) folded into the query projection
                "Wq": np.ascontiguousarray(Wq[sl]) * 0.125,
                "bq": np.ascontiguousarray(bq[sl]) * 0.125,
            }
        )
    return in_maps


def gather_out(results):
    out = np.zeros((B, H * HD), np.float32)
    for c in range(8):
        b, g = divmod(c, 2)
        # compensate the folded 1/8 query scale
        out[b, g * DG : (g + 1) * DG] = results[c]["out"] * 8.0
    return out


def run_sharded(inputs, trace=False, **kwargs):
    from concourse.bass_utils import run_bass_kernel_spmd

    nc = _program()
    res = run_bass_kernel_spmd(
        nc, make_in_maps(inputs), core_ids=list(range(8)), trace=trace, **kwargs
    )
    return gather_out(res.results), res


def kernel(**inputs):
    out, _ = run_sharded(inputs)
    return out

